# revision 8
# baseline (speedup 1.0000x reference)
"""YOLO-style detection decode (nms_detection) on 8 trn2 NeuronCores.

Data-parallel over batch (64 -> 8 images/core). Per core, per scale the
input [8, 255, H, W] is host-rearranged to [255, 8*H*W] (channel-major,
flat cell axis). The device kernel:

  - DMAs channel-permuted strips into SBUF:  [box15 | cls113] and [cls127]
    where box rows are (k, a) k in {conf,tx,ty,tw,th}, cls rows are
    anchor-major class logits (3 x 80).
  - PE-transposes 128-cell chunks into PSUM -> [cell, 255ch] layout.
  - DVE reduce_max over the 80 class cols per anchor -> m (exact f32 max).
  - m is split into 3 bf16 terms (exact), PE-transposed, and a tiny bf16
    matmul subtracts m from every class logit (exact by Sterbenz) and a
    second K=1 matmul adds (79-c)*2^-31. The winner's value is then
    exactly (79-argmax)*2^-31 >= 0 and every loser stays < 0, so a second
    DVE reduce_max recovers argmax exactly (incl. first-index ties).
  - conf = 1/(1+exp(-logit)) (Exp table set only), cx/cy via fused
    scalar_tensor_tensor with host-precomputed grid offsets, w/h via
    exp * anchors/416, mask = logit > 0 applied by multiply.
  - Output rows [cell, 18] (3 anchors x 6) DMA'd contiguously.
"""

from contextlib import ExitStack

import numpy as np

import concourse.bass as bass
import concourse.tile as tile
from concourse import bacc, mybir
from concourse.bass_utils import run_bass_kernel_spmd

N_CORES = 8
B = 64
B_PER = B // N_CORES
CASE = 416.0
# (tag, H, t)
SCALES = [("52", 52, 8.0), ("26", 26, 16.0), ("13", 13, 32.0)]
CHUNK = 128
GRP = 4  # chunks per group (psum tile = 4 banks)
F32 = mybir.dt.float32
BF16 = mybir.dt.bfloat16
AX = mybir.AxisListType
OP = mybir.AluOpType
AF = mybir.ActivationFunctionType
IOTA_SCALE = 2.0 ** -31


def _cells(h):
    return B_PER * h * h


def _nchunks(h):
    return (_cells(h) + CHUNK - 1) // CHUNK


def _gxy_section(h, t):
    """[128, 2*nchunks] f32: per chunk j, cols (2j, 2j+1) = (gx, gy)*t/CASE
    for the 128 cells j*128+p (p = partition)."""
    n = _cells(h)
    nch = _nchunks(h)
    cells = np.arange(nch * CHUNK)
    s = cells % (h * h)  # position within image
    gx = (s % h).astype(np.float64) * t / CASE
    gy = (s // h).astype(np.float64) * t / CASE
    gx[cells >= n] = 0.0
    gy[cells >= n] = 0.0
    out = np.zeros((CHUNK, 2 * nch), np.float32)
    for j in range(nch):
        out[:, 2 * j] = gx[j * CHUNK:(j + 1) * CHUNK]
        out[:, 2 * j + 1] = gy[j * CHUNK:(j + 1) * CHUNK]
    return out


def _consts():
    import ml_dtypes
    sel9 = np.zeros((128, 240), np.float32)
    for k in range(4):
        for r in range(9):
            a = r % 3
            sel9[32 * k + r, 80 * a:80 * a + 80] = -1.0
    iotam = np.zeros((1, 240), np.float32)
    iotam[0, :] = (79.0 - (np.arange(240) % 80)) * IOTA_SCALE
    onesb = np.ones((1, 128), np.float32)
    iden = np.eye(128, dtype=np.float32)
    gxy = np.concatenate([_gxy_section(h, t) for _, h, t in SCALES], axis=1)
    bf = ml_dtypes.bfloat16
    return {
        "sel9": sel9.astype(bf),
        "iotam": iotam.astype(bf),
        "onesb": onesb.astype(bf),
        "iden": iden,
        "gxy": gxy,
    }


_CONSTS = _consts()


def _emit_scale(nc, tc, ctx, pools, sb, xr, oX, h, t, gxy_off):
    """Emit all groups for one scale."""
    n = _cells(h)
    nch = _nchunks(h)
    ngrp = (nch + GRP - 1) // GRP
    k = float(t / CASE)

    (p_ina, p_inb, p_ps, p_m, p_sm, p_mt, p_out) = pools

    for g in range(ngrp):
        j0 = g * GRP
        gc = min(GRP, nch - j0)  # chunks in this group
        c0 = j0 * CHUNK
        w = min(GRP * CHUNK, n - c0)  # valid cells in window

        # ---- input strips (channel-permuted) ----
        in_a = p_ina.tile([128, GRP * CHUNK], F32, tag="in_a")
        in_b = p_inb.tile([128, GRP * CHUNK], F32, tag="in_b")
        xr_ar = xr.rearrange("(a r) n -> a r n", a=3)
        for kk in range(5):
            nc.sync.dma_start(in_a[3 * kk:3 * kk + 3, 0:w],
                              xr_ar[:, kk, c0:c0 + w])
        nc.sync.dma_start(in_a[15:95, 0:w], xr[5:85, c0:c0 + w])
        nc.sync.dma_start(in_a[95:128, 0:w], xr[90:123, c0:c0 + w])
        nc.sync.dma_start(in_b[0:47, 0:w], xr[123:170, c0:c0 + w])
        nc.sync.dma_start(in_b[47:127, 0:w], xr[175:255, c0:c0 + w])

        # ---- PSUM group tile + transposes ----
        ps = p_ps.tile([128, 4 * 512], F32, tag="ps")
        pg = ps[:].rearrange("p (g x) -> p g x", g=4)
        ncs = []
        for jj in range(gc):
            ncj = min(CHUNK, w - jj * CHUNK)
            ncs.append(ncj)
            xa = in_a[:, jj * CHUNK:jj * CHUNK + ncj]
            xb = in_b[0:127, jj * CHUNK:jj * CHUNK + ncj]
            if ncj < CHUNK:
                nc.vector.memset(ps[:, jj * 512:jj * 512 + 255], 0.0)
            nc.tensor.transpose(ps[0:ncj, jj * 512:jj * 512 + 128],
                                xa, sb["iden"][:, :])
            nc.tensor.matmul(ps[0:ncj, jj * 512 + 128:jj * 512 + 255],
                             xb, sb["iden"][0:127, 0:127],
                             is_transpose=True, start=False, stop=True,
                             skip_group_check=True)

        cls4 = pg[:, 0:gc, 15:255].rearrange("p g (a c) -> p g a c", a=3)
        conf_ps = pg[:, 0:gc, 0:3]
        txy_ps = pg[:, 0:gc, 3:9].rearrange("p g (k a) -> p g k a", k=2)
        twh_ps = pg[:, 0:gc, 9:15].rearrange("p g (k a) -> p g k a", k=2)

        # ---- scan 1: exact class max ----
        m_sb = p_m.tile([128, 12], F32, tag="m_sb")
        m_v = m_sb[:].rearrange("p (g a) -> p g a", g=4)[:, 0:gc, :]
        nc.vector.tensor_reduce(m_v, cls4, axis=AX.X, op=OP.max)

        # ---- bf16 3-term split of m (exact), on gpsimd ----
        hb = p_m.tile([128, 12], BF16, tag="hb")
        hb2 = p_m.tile([128, 12], BF16, tag="hb2")
        r1 = p_m.tile([128, 12], F32, tag="r1")
        msp = p_sm.tile([128, 128], F32, tag="msp")
        nc.gpsimd.memset(msp[:, :], 0.0)
        mspv = msp[:].rearrange("p (g r) -> p g r", g=4)
        hb_v = hb[:].rearrange("p (g a) -> p g a", g=4)[:, 0:gc, :]
        hb2_v = hb2[:].rearrange("p (g a) -> p g a", g=4)[:, 0:gc, :]
        r1_v = r1[:].rearrange("p (g a) -> p g a", g=4)[:, 0:gc, :]
        nc.gpsimd.tensor_copy(hb_v, m_v)
        nc.gpsimd.tensor_copy(mspv[:, 0:gc, 0:3], hb_v)
        nc.gpsimd.tensor_tensor(r1_v, m_v, hb_v, op=OP.subtract)
        nc.gpsimd.tensor_copy(hb2_v, r1_v)
        nc.gpsimd.tensor_copy(mspv[:, 0:gc, 3:6], hb2_v)
        nc.gpsimd.tensor_tensor(mspv[:, 0:gc, 6:9], r1_v, hb2_v,
                                op=OP.subtract)

        # ---- transpose m-split into psum spare, copy to SBUF bf16 ----
        # (two halves so per-chunk stationary rows sit at base 0 / 32)
        mts = []
        for hh in range((gc + 1) // 2):
            nc.tensor.matmul(ps[0:64, hh * 512 + 256:hh * 512 + 384],
                             msp[:, 64 * hh:64 * hh + 64],
                             sb["iden"][:, :],
                             is_transpose=True, start=False, stop=True,
                             skip_group_check=True)
            mt_t = p_mt.tile([64, 128], BF16, tag=f"mtsb{hh}")
            nc.scalar.copy(mt_t[:, :], ps[0:64, hh * 512 + 256:hh * 512 + 384])
            mts.append(mt_t)

        # ---- recenter: psum_cls += -m, then += iota (exact ordering) ----
        for jj in range(gc):
            out_cls = ps[:, jj * 512 + 15:jj * 512 + 255]
            bp = 32 * (jj % 2)
            nc.tensor.matmul(out_cls, mts[jj // 2][bp:bp + 9, :],
                             sb["sel9"][bp:bp + 9, :],
                             start=False, stop=True,
                             skip_group_check=True)
            nc.tensor.matmul(out_cls, sb["onesb"][:, :], sb["iotam"][:, :],
                             start=False, stop=True, skip_group_check=True)

        # ---- scan 2: argmax via reduce_max of recentered values ----
        idx_sb = p_m.tile([128, 12], F32, tag="idx_sb")
        idx_v = idx_sb[:].rearrange("p (g a) -> p g a", g=4)[:, 0:gc, :]
        nc.vector.tensor_reduce(idx_v, cls4, axis=AX.X, op=OP.max)

        # ---- decode ----
        out4 = p_out.tile([128, GRP * 18], F32, tag="out4")
        o4 = out4[:].rearrange("p (g a s) -> p g a s", g=4, a=3)
        o4t = out4[:].rearrange("p (g a s) -> p g s a", g=4, a=3)

        # conf = 1 / (1 + exp(-logit))
        econf = p_m.tile([128, 12], F32, tag="econf")
        e_v = econf[:].rearrange("p (g a) -> p g a", g=4)[:, 0:gc, :]
        nc.scalar.activation(e_v, conf_ps, AF.Exp, scale=-1.0)
        ep1 = p_m.tile([128, 12], F32, tag="ep1")
        e1_v = ep1[:].rearrange("p (g a) -> p g a", g=4)[:, 0:gc, :]
        nc.gpsimd.tensor_scalar(e1_v, e_v, 1.0, None, op0=OP.add)
        nc.vector.reciprocal(o4t[:, 0:gc, 0:1, :].squeeze(2), e1_v)

        # cx, cy = logit * (t/CASE) + g*(t/CASE)
        gxy_ap = sb["gxy"][:, gxy_off + 2 * j0:gxy_off + 2 * j0 + 2 * gc]
        gxy_r = gxy_ap.rearrange("p (g k) -> p g k", k=2)
        for kk in range(2):  # 0 -> cx (tx+gx), 1 -> cy (ty+gy)
            g_v = gxy_r[:, :, kk:kk + 1].broadcast_to([128, gc, 3])
            src = pg[:, 0:gc, 3 + 3 * kk:6 + 3 * kk]
            dst = o4t[:, 0:gc, 1 + kk:2 + kk, :].squeeze(2)
            nc.vector.scalar_tensor_tensor(dst, src, k, g_v,
                                           op0=OP.mult, op1=OP.add)

        # w, h = exp(logit) * anchors/CASE
        twh = p_m.tile([128, 24], F32, tag="twh")
        twh_v = twh[:].rearrange("p (g k a) -> p g k a", g=4, k=2)[:, 0:gc]
        nc.scalar.activation(twh_v, twh_ps, AF.Exp)
        anch_v = sb["anch"][:, 0:6].rearrange("p (k a) -> p k a", k=2) \
            .unsqueeze(1).broadcast_to([128, gc, 2, 3])
        nc.vector.tensor_tensor(o4t[:, 0:gc, 3:5, :], twh_v, anch_v,
                                op=OP.mult)

        # cls = 79 - idxraw * 2^31
        nc.scalar.activation(o4t[:, 0:gc, 5:6, :].squeeze(2), idx_v,
                             AF.Copy, bias=79.0, scale=-(2.0 ** 31))

        # mask = logit > 0, applied multiplicatively to all 6 outputs
        msk = p_m.tile([128, 12], F32, tag="msk")
        msk_v = msk[:].rearrange("p (g a) -> p g a", g=4)[:, 0:gc, :]
        nc.vector.tensor_scalar(msk_v, conf_ps, 0.0, None, op0=OP.is_gt)
        o4g = o4[:, 0:gc]
        msk_b = msk_v.unsqueeze(3).broadcast_to([128, gc, 3, 6])
        nc.vector.tensor_tensor(o4g, o4g, msk_b, op=OP.mult)

        # ---- store ----
        for jj in range(gc):
            ncj = ncs[jj]
            r0 = c0 + jj * CHUNK
            nc.sync.dma_start(oX[r0:r0 + ncj, :],
                              out4[0:ncj, 18 * jj:18 * jj + 18])


def build():
    nc = bacc.Bacc("TRN2", target_bir_lowering=False, debug=False,
                   num_devices=N_CORES)
    xr, oX = {}, {}
    for tag, h, _ in SCALES:
        xr[tag] = nc.dram_tensor(f"x{tag}", [255, _cells(h)], F32,
                                 kind="ExternalInput").ap()
        oX[tag] = nc.dram_tensor(f"o{tag}", [_cells(h), 18], F32,
                                 kind="ExternalOutput").ap()
    dconst = {}
    shapes = {"sel9": ([128, 240], BF16), "iotam": ([1, 240], BF16),
              "onesb": ([1, 128], BF16), "iden": ([128, 128], F32),
              "gxy": ([128, _CONSTS["gxy"].shape[1]], F32),
              "anch": ([128, 18], F32)}
    for name, (shp, dt) in shapes.items():
        dconst[name] = nc.dram_tensor(name, shp, dt,
                                      kind="ExternalInput").ap()

    with tile.TileContext(nc) as tc:
        with ExitStack() as ctx:
            p_c = ctx.enter_context(tc.tile_pool(name="consts", bufs=1))
            p_ina = ctx.enter_context(tc.tile_pool(name="inpa", bufs=4))
            p_inb = ctx.enter_context(tc.tile_pool(name="inpb", bufs=4))
            p_ps = ctx.enter_context(
                tc.tile_pool(name="ps", bufs=2, space="PSUM"))
            p_m = ctx.enter_context(tc.tile_pool(name="small", bufs=3))
            p_sm = ctx.enter_context(tc.tile_pool(name="msp", bufs=3))
            p_mt = ctx.enter_context(tc.tile_pool(name="mt", bufs=3))
            p_out = ctx.enter_context(tc.tile_pool(name="out", bufs=4))

            sb = {}
            for name, (shp, dt) in shapes.items():
                t_ = p_c.tile(shp, dt, tag=name)
                nc.sync.dma_start(t_[:], dconst[name])
                sb[name] = t_[:]

            pools = (p_ina, p_inb, p_ps, p_m, p_sm, p_mt, p_out)
            gxy_off = 0
            anch_off = 0
            for tag, h, t in SCALES:
                sbs = dict(sb)
                sbs["anch"] = sb["anch"][:, anch_off:anch_off + 6]
                _emit_scale(nc, tc, ctx, pools, sbs, xr[tag], oX[tag],
                            h, t, gxy_off)
                gxy_off += 2 * _nchunks(h)
                anch_off += 6
    nc.compile()
    return nc


_NC = None


def _get_nc():
    global _NC
    if _NC is None:
        _NC = build()
    return _NC


def kernel(out13, out26, out52, anchors13, anchors26, anchors52):
    nc = _get_nc()
    xs = {"13": np.asarray(out13), "26": np.asarray(out26),
          "52": np.asarray(out52)}
    anchors = {"13": np.asarray(anchors13), "26": np.asarray(anchors26),
               "52": np.asarray(anchors52)}

    anch = np.zeros((128, 18), np.float32)
    off = 0
    for tag, h, _ in SCALES:
        a = anchors[tag].astype(np.float64) / CASE
        for kk in range(2):
            for aa in range(3):
                anch[:, off + kk * 3 + aa] = a[aa, kk]
        off += 6

    in_maps = []
    for i in range(N_CORES):
        m = {}
        for tag, h, _ in SCALES:
            x = xs[tag][i * B_PER:(i + 1) * B_PER]
            m[f"x{tag}"] = np.ascontiguousarray(
                x.transpose(1, 0, 2, 3).reshape(255, -1))
        m["anch"] = anch
        for name in ("sel9", "iotam", "onesb", "iden", "gxy"):
            m[name] = _CONSTS[name]
        in_maps.append(m)

    res = run_bass_kernel_spmd(nc, in_maps, list(range(N_CORES))).results

    parts = []
    for tag, h, _ in SCALES[::-1]:  # output order: 13, 26, 52
        for i in range(N_CORES):
            parts.append(res[i][f"o{tag}"].reshape(-1, 6))
    return np.concatenate(parts, axis=0)


# revision 13
# speedup vs baseline: 3.5226x; 3.5226x over previous
"""YOLO-style detection decode (nms_detection) on 8 trn2 NeuronCores.

Data-parallel over batch (64 -> 8 images/core). All per-core inputs are
packed into ONE flat f32 DRAM tensor (x52|x26|x13 in natural [b,ch,s]
order, then small constants) and the result is ONE [28392, 18] f32
tensor (cells x (3 anchors x 6)), reassembled on the host. One input +
one output minimizes the large per-tensor dispatch overhead of the
execution path.

Device pipeline per 4-chunk group (chunk = 128 cells):
  - segment DMAs load [128ch, cells] strips (raw channel order).
  - PE transposes chunks into PSUM -> [cell, 255ch].
  - DVE reduce_max over the 80 class cols per anchor -> m (exact).
  - PE transposes m; an fp32 K=3 matmul subtracts m from the class
    logits (exact: Sterbenz near the max) and a K=1 matmul adds
    (79-c)*2^-31. The winner's value is then exactly
    (79-argmax)*2^-31 >= 0 while every loser stays < 0, so a second
    DVE reduce_max recovers argmax exactly (incl. first-index ties,
    matching jnp.argmax).
  - decode: conf = sigmoid (ACT), cx/cy fused scalar_tensor_tensor with
    host grid offsets, w/h = exp * anchors/416, mask = (logit > 0)
    applied multiplicatively (fused is_gt*mult per anchor).
"""

import os
from contextlib import ExitStack

import numpy as np

import concourse.bass as bass
import concourse.tile as tile
from concourse import bacc, mybir
from concourse.bass_utils import run_bass_kernel_spmd

N_CORES = 8
B = 64
B_PER = B // N_CORES
CASE = 416.0
SCALES = [("52", 52, 8.0), ("26", 26, 16.0), ("13", 13, 32.0)]
CHUNK = 128
GRP = 4
F32 = mybir.dt.float32
AX = mybir.AxisListType
OP = mybir.AluOpType
AF = mybir.ActivationFunctionType
IOTA_SCALE = 2.0 ** -31


def _cells(h):
    return B_PER * h * h


def _nchunks(h):
    return (_cells(h) + CHUNK - 1) // CHUNK


def _gxy_section(h, t):
    n = _cells(h)
    nch = _nchunks(h)
    cells = np.arange(nch * CHUNK)
    s = cells % (h * h)
    gx = (s % h).astype(np.float64) * t / CASE
    gy = (s // h).astype(np.float64) * t / CASE
    gx[cells >= n] = 0.0
    gy[cells >= n] = 0.0
    out = np.zeros((CHUNK, 2 * nch), np.float32)
    for j in range(nch):
        out[:, 2 * j] = gx[j * CHUNK:(j + 1) * CHUNK]
        out[:, 2 * j + 1] = gy[j * CHUNK:(j + 1) * CHUNK]
    return out


def _consts():
    # raw channel order: anchor a's class cols at 85a+5 .. 85a+85
    sel3 = np.zeros((128, 255), np.float32)
    for q in range(4):
        for a in range(3):
            sel3[32 * q + a, 85 * a + 5:85 * a + 85] = -1.0
    iotam = np.zeros((1, 255), np.float32)
    for a in range(3):
        iotam[0, 85 * a + 5:85 * a + 85] = \
            (79.0 - np.arange(80)) * IOTA_SCALE
    onesb = np.ones((1, 128), np.float32)
    iden = np.eye(128, dtype=np.float32)
    gxy = np.concatenate([_gxy_section(h, t) for _, h, t in SCALES], axis=1)
    return {
        "gxy": gxy.astype(np.float32),
        "iden": iden,
        "sel3": sel3,
        "iotam": iotam,
        "onesb": onesb,
    }


_CONSTS = _consts()

# packed input layout (f32 elements, per core)
_X_OFF = {}
_off = 0
for _tag, _h, _t in SCALES:
    _X_OFF[_tag] = _off
    _off += B_PER * 255 * _h * _h
_CONST_OFF = {}
for _name in ("gxy", "iden", "sel3", "iotam", "onesb"):
    _CONST_OFF[_name] = _off
    _off += _CONSTS[_name].size
_CONST_OFF["anch"] = _off
_off += 128 * 18
TOTAL_IN = _off

_O_OFF = {}
_off = 0
for _tag, _h, _t in SCALES:
    _O_OFF[_tag] = _off
    _off += _cells(_h)
TOTAL_OUT_ROWS = _off  # 28392


def _a85(ap_pgx, lo, width=1):
    """[128, gc, 3(anchor), width] view of box channel `lo` from a
    [128, gc, 512] psum group view (channel stride 85)."""
    v = ap_pgx[:, :, 0:255].rearrange("p g (a r) -> p g a r", a=3, r=85)
    return v[:, :, :, lo:lo + width]


def _emit_scale(nc, tc, ctx, pools, sb, xin, oX, h, t, tag, gxy_off):
    ST = int(os.environ.get("KSTAGE", "9"))
    n = _cells(h)
    hw = h * h
    nch = _nchunks(h)
    ngrp = (nch + GRP - 1) // GRP
    k = float(t / CASE)
    (p_ina, p_inb, p_ps, p_m, p_mt, p_out) = pools

    xoff = _X_OFF[tag]
    xr3 = xin[xoff:xoff + B_PER * 255 * hw] \
        .rearrange("(b c s) -> c b s", b=B_PER, c=255)

    def seg_dma(dst_tile, nrows, src0, c0, w):
        done = 0
        while done < w:
            cell = c0 + done
            b = cell // hw
            s = cell % hw
            span = min(w - done, hw - s)
            nc.sync.dma_start(dst_tile[0:nrows, done:done + span],
                              xr3[src0:src0 + nrows, b, s:s + span])
            done += span

    for g in range(ngrp):
        j0 = g * GRP
        gc = min(GRP, nch - j0)
        c0 = j0 * CHUNK
        w = min(GRP * CHUNK, n - c0)

        in_a = p_ina.tile([128, GRP * CHUNK], F32, tag="in_a")
        in_b = p_inb.tile([128, GRP * CHUNK], F32, tag="in_b")
        seg_dma(in_a, 128, 0, c0, w)
        seg_dma(in_b, 127, 128, c0, w)

        ps = p_ps.tile([128, 4 * 512], F32, tag="ps")
        pg = ps[:].rearrange("p (g x) -> p g x", g=4)[:, 0:gc, :]
        ncs = []
        for jj in range(gc):
            ncj = min(CHUNK, w - jj * CHUNK)
            ncs.append(ncj)
            if ncj < CHUNK:
                nc.vector.memset(ps[:, jj * 512:jj * 512 + 255], 0.0)
            nc.tensor.transpose(ps[0:ncj, jj * 512:jj * 512 + 128],
                                in_a[:, jj * CHUNK:jj * CHUNK + ncj],
                                sb["iden"])
            nc.tensor.matmul(ps[0:ncj, jj * 512 + 128:jj * 512 + 255],
                             in_b[0:127, jj * CHUNK:jj * CHUNK + ncj],
                             sb["iden"][0:127, 0:127],
                             is_transpose=True, start=False, stop=True,
                             skip_group_check=True)

        cls_ap = _a85(pg, 5, 80)          # [128, gc, 3, 80]
        conf_ap = _a85(pg, 0).squeeze(3)  # [128, gc, 3]

        # ---- scan 1: exact class max (into cols 32g + a) ----
        m_sb = p_m.tile([128, 128], F32, tag="m_sb")
        nc.vector.memset(m_sb[:, :], 0.0)
        m_v = m_sb[:].rearrange("p (g r) -> p g r", g=4)[:, 0:gc, 0:3]
        if ST >= 2:
            nc.vector.tensor_reduce(m_v, cls_ap, axis=AX.X, op=OP.max)

        # ---- transpose m into psum spare (two halves: bases 0/32) ----
        mts = []
        for hh in range((gc + 1) // 2 if ST >= 4 else 0):
            nc.tensor.matmul(ps[0:64, hh * 512 + 256:hh * 512 + 384],
                             m_sb[:, 64 * hh:64 * hh + 64],
                             sb["iden"][0:128, 0:128],
                             is_transpose=True, start=False, stop=True,
                             skip_group_check=True)
            mt_t = p_mt.tile([64, 128], F32, tag=f"mtsb{hh}")
            nc.scalar.copy(mt_t[:, :],
                           ps[0:64, hh * 512 + 256:hh * 512 + 384])
            mts.append(mt_t)

        # ---- recenter: cls += -m, then += iota (separate accumulates) --
        for jj in range(gc if ST >= 5 else 0):
            out_cls = ps[:, jj * 512:jj * 512 + 255]
            bp = 32 * (jj % 2)
            nc.tensor.matmul(out_cls, mts[jj // 2][bp:bp + 3, :],
                             sb["sel3"][bp:bp + 3, :],
                             start=False, stop=True, skip_group_check=True)
            nc.tensor.matmul(out_cls, sb["onesb"], sb["iotam"],
                             start=False, stop=True, skip_group_check=True)

        # ---- scan 2: argmax ----
        idx_sb = p_m.tile([128, 12], F32, tag="idx_sb")
        idx_v = idx_sb[:].rearrange("p (g a) -> p g a", g=4)[:, 0:gc, :]
        if ST >= 6:
            nc.vector.tensor_reduce(idx_v, cls_ap, axis=AX.X, op=OP.max)
        else:
            nc.vector.memset(idx_sb[:, :], 0.0)

        # ---- decode ----
        out4 = p_out.tile([128, GRP * 18], F32, tag="out4")
        if ST < 7:
            nc.vector.memset(out4[:, :], 0.0)
        o4 = out4[:].rearrange("p (g a s) -> p g a s", g=4, a=3)
        o4t = out4[:].rearrange("p (g a s) -> p g s a", g=4, a=3)

        if ST >= 7:
            nc.scalar.activation(o4t[:, 0:gc, 0:1, :].squeeze(2), conf_ap,
                                 AF.Sigmoid)

            gxy_ap = sb["gxy"][:, gxy_off + 2 * j0:gxy_off + 2 * j0 + 2 * gc]
            gxy_r = gxy_ap.rearrange("p (g q) -> p g q", q=2)
            for kk in range(2):
                g_v = gxy_r[:, :, kk:kk + 1].broadcast_to([128, gc, 3])
                src = _a85(pg, 1 + kk).squeeze(3)
                dst = o4t[:, 0:gc, 1 + kk:2 + kk, :].squeeze(2)
                nc.vector.scalar_tensor_tensor(dst, src, k, g_v,
                                               op0=OP.mult, op1=OP.add)

            twh = p_m.tile([128, 24], F32, tag="twh")
            twh_v = twh[:].rearrange("p (g q a) -> p g q a", g=4, q=2)
            for kk in range(2):
                nc.scalar.activation(
                    twh_v[:, 0:gc, kk:kk + 1, :].squeeze(2),
                    _a85(pg, 3 + kk).squeeze(3), AF.Exp)
            anch_v = sb["anch"].rearrange("p (q a) -> p q a", q=2) \
                .unsqueeze(1).broadcast_to([128, gc, 2, 3])
            nc.vector.tensor_tensor(o4t[:, 0:gc, 3:5, :],
                                    twh_v[:, 0:gc], anch_v, op=OP.mult)

            nc.scalar.activation(o4t[:, 0:gc, 5:6, :].squeeze(2), idx_v,
                                 AF.Copy, bias=79.0, scale=-(2.0 ** 31))

            for a in range(3):
                cb = conf_ap[:, :, a:a + 1].broadcast_to([128, gc, 6])
                dst = o4[:, 0:gc, a, :]
                nc.vector.scalar_tensor_tensor(dst, cb, 0.0, dst,
                                               op0=OP.is_gt, op1=OP.mult)

        for jj in range(gc):
            ncj = ncs[jj]
            r0 = _O_OFF[tag] + c0 + jj * CHUNK
            nc.sync.dma_start(oX[r0:r0 + ncj, :],
                              out4[0:ncj, 18 * jj:18 * jj + 18])


def build():
    nc = bacc.Bacc("TRN2", target_bir_lowering=False, debug=False,
                   num_devices=N_CORES)
    xin = nc.dram_tensor("xin", [TOTAL_IN], F32, kind="ExternalInput").ap()
    oX = nc.dram_tensor("out", [TOTAL_OUT_ROWS, 18], F32,
                        kind="ExternalOutput").ap()

    with tile.TileContext(nc) as tc:
        with ExitStack() as ctx:
            p_c = ctx.enter_context(tc.tile_pool(name="consts", bufs=1))
            p_ina = ctx.enter_context(tc.tile_pool(name="inpa", bufs=4))
            p_inb = ctx.enter_context(tc.tile_pool(name="inpb", bufs=4))
            p_ps = ctx.enter_context(
                tc.tile_pool(name="ps", bufs=2, space="PSUM"))
            p_m = ctx.enter_context(tc.tile_pool(name="small", bufs=3))
            p_mt = ctx.enter_context(tc.tile_pool(name="mt", bufs=3))
            p_out = ctx.enter_context(tc.tile_pool(name="out", bufs=4))

            shapes = {"gxy": [128, _CONSTS["gxy"].shape[1]],
                      "iden": [128, 128], "sel3": [128, 255],
                      "iotam": [1, 255], "onesb": [1, 128],
                      "anch": [128, 18]}
            sb = {}
            for name, shp in shapes.items():
                t_ = p_c.tile(shp, F32, tag=name)
                size = shp[0] * shp[1]
                src = xin[_CONST_OFF[name]:_CONST_OFF[name] + size] \
                    .rearrange("(p f) -> p f", p=shp[0])
                nc.sync.dma_start(t_[:], src)
                sb[name] = t_[:]
            anch_t = sb["anch"]

            pools = (p_ina, p_inb, p_ps, p_m, p_mt, p_out)
            for _rep in range(int(os.environ.get("KREP", "1"))):
                gxy_off = 0
                anch_off = 0
                for tag, h, t in SCALES:
                    sbs = dict(sb)
                    sbs["anch"] = anch_t[:, anch_off:anch_off + 6]
                    _emit_scale(nc, tc, ctx, pools, sbs, xin, oX, h, t,
                                tag, gxy_off)
                    gxy_off += 2 * _nchunks(h)
                    anch_off += 6
    nc.compile()
    return nc


_NC = None


def _get_nc():
    global _NC
    if _NC is None:
        _NC = build()
    return _NC


def _make_anch(anchors):
    anch = np.zeros((128, 18), np.float32)
    off = 0
    for tag, h, _ in SCALES:
        a = anchors[tag].astype(np.float64) / CASE
        for kk in range(2):
            for aa in range(3):
                anch[:, off + kk * 3 + aa] = a[aa, kk]
        off += 6
    return anch


def _pack_core(xs, anch):
    parts = [np.asarray(xs["52"]).ravel(), np.asarray(xs["26"]).ravel(),
             np.asarray(xs["13"]).ravel(),
             _CONSTS["gxy"].ravel(), _CONSTS["iden"].ravel(),
             _CONSTS["sel3"].ravel(), _CONSTS["iotam"].ravel(),
             _CONSTS["onesb"].ravel(), anch.ravel()]
    out = np.concatenate(parts)
    assert out.size == TOTAL_IN and out.dtype == np.float32
    return out


def kernel(out13, out26, out52, anchors13, anchors26, anchors52):
    nc = _get_nc()
    xs_all = {"13": np.asarray(out13), "26": np.asarray(out26),
              "52": np.asarray(out52)}
    anchors = {"13": np.asarray(anchors13), "26": np.asarray(anchors26),
               "52": np.asarray(anchors52)}
    anch = _make_anch(anchors)

    in_maps = []
    for i in range(N_CORES):
        xs = {tag: xs_all[tag][i * B_PER:(i + 1) * B_PER]
              for tag, _, _ in SCALES}
        in_maps.append({"xin": _pack_core(xs, anch)})

    res = run_bass_kernel_spmd(nc, in_maps, list(range(N_CORES))).results

    parts = []
    for tag, h, _ in SCALES[::-1]:  # output order: 13, 26, 52
        o0 = _O_OFF[tag]
        for i in range(N_CORES):
            parts.append(res[i]["out"][o0:o0 + _cells(h)].reshape(-1, 6))
    return np.concatenate(parts, axis=0)
